# revision 23
# baseline (speedup 1.0000x reference)
"""DBRX attention block (B=1, T=2048, D=6144, 48 q heads / 8 kv heads, RoPE,
clamp, causal) as a Bass/Tile kernel on 8 Trainium2 NeuronCores.

Sharding: tensor-parallel over heads. Core c owns q heads [6c, 6c+6) and kv
head c (GQA groups align exactly: q head i uses kv head i//6).

Pipeline per core (feature-major [feature, T] layout so every matmul is a
clean PE op at full rate; fp16 projections — fp8 was measured at ~5e-2
rel err, over the gate):
  - x ships SHARDED (768 rows of xT per core, fp16) and is all-gathered
    on-device over NeuronLink into a Shared-address-space HBM buffer,
  - qkvT = wqkvT.T @ xT in fp16 (PSUM accumulates fp32),
  - RoPE via a pair-swap permutation matmul + DVE combine (fp32r),
  - scoresT[k, q] = kT.T @ qT (fp32r, full rate at 512-wide); exp on ACT
    (no max-subtraction: scores bounded by the clamp, fp32 range holds),
  - attnT = v @ expT (fp32r); softmax denominator via ones-vector matmul,
    applied with a reciprocal + DMA partition-broadcast,
  - outT = w_o.T @ attnT (fp16), partials ReduceScattered (Shared output)
    so each core ships back only its [768, 2048] fp16 shard.

reps>1 unrolls the whole body (incl. collectives) N times back-to-back in
one program for steady-state marginal-cost timing; see build_program.
"""

import math
import sys
from contextlib import ExitStack

import numpy as np

for _p in ("/opt/trn_rl_repo", "/root/.axon_site/_ro/trn_rl_repo"):
    if _p not in sys.path:
        sys.path.append(_p)

import concourse.bass as bass
import concourse.tile as tile
from concourse import bacc, bass_isa, mybir
from concourse.bass_utils import run_bass_kernel_spmd

F16 = mybir.dt.float16
F32 = mybir.dt.float32
F32R = mybir.dt.float32r
MULT = mybir.AluOpType.mult
ADD = mybir.AluOpType.add
MAX = mybir.AluOpType.max
MIN = mybir.AluOpType.min
EXP = mybir.ActivationFunctionType.Exp

N_CORES = 8
D_MODEL = 6144
N_HEADS = 48
N_KV_HEADS = 8
HEAD_DIM = 128
NQH = N_HEADS // N_CORES          # q heads per core = 6
T = 2048
CLAMP = 8.0
ROPE_BASE = 500000.0
SCALE = 1.0 / math.sqrt(HEAD_DIM)
MASK_NEG = -30000.0               # fp16-safe; SCALE*MASK_NEG << -90 => exp = 0
XSH = D_MODEL // N_CORES          # x shard rows per core = 768


def _chunk_plan(kd, chunk):
    """Chunk sizes: small leading chunks so the PE starts early."""
    plan = []
    rest = kd
    if kd > 8:
        plan = [1, 2, 3]
        rest = kd - 6
    while rest > 0:
        c = min(chunk, rest)
        plan.append(c)
        rest -= c
    return plan


def _pk_offsets(d_model, t, nqh, n_cores, use_cc):
    """Element offsets of each logical tensor inside the flat fp16 operand."""
    xsh = d_model // n_cores
    nf = nqh + 2
    qkv_cols = nf * 128
    off = {}
    cur = 0
    off["xs"] = cur
    cur += (xsh if use_cc else d_model) * t
    off["wq"] = cur
    cur += d_model * qkv_cols
    off["wo"] = cur
    cur += nqh * 128 * d_model
    off["cos"] = cur
    cur += 128 * t
    off["sin"] = cur
    cur += 128 * t
    off["mask"] = cur
    cur += 4 * 128 * 512
    off["pswap"] = cur
    cur += 128 * 128
    off["ident"] = cur
    cur += 128 * 128
    off["ones"] = cur
    cur += 128
    return off, cur


def build_program(d_model=D_MODEL, t=T, nqh=NQH, chunk=6, n_cores=N_CORES,
                  use_cc=True, reps=1, _skip=()):
    """Build the per-core Bass program. Returns the compiled Bacc handle.

    use_cc=False builds a single-core variant (full xT input, full partial
    output, no collectives) for CoreSim numerical checking.

    reps>1 unrolls the whole kernel body (phases 1-3 including the
    collectives) N times back-to-back inside one program execution, for
    steady-state device timing: per-invocation time is the marginal cost
    of one extra rep. Each rep re-reads x and all weights from DRAM and
    re-runs the AllGather/ReduceScatter, so one rep is exactly one honest
    kernel invocation; only the tiny input-independent constant staging
    (rope/mask tables, ~1.3 MB) is hoisted out of the loop.
    """
    kd = d_model // 128            # contraction tiles for qkv projection
    tq = t // 512                  # 512-wide T quads
    tb = t // 128                  # 128-wide T blocks
    nf = nqh + 2                   # feature tiles: q heads + k + v
    qkv_cols = nf * 128
    xsh = d_model // n_cores       # x shard rows

    off, pk_elems = _pk_offsets(d_model, t, nqh, n_cores, use_cc)

    nc = bacc.Bacc("TRN2", target_bir_lowering=False, debug=False,
                   num_devices=n_cores)

    pk = nc.dram_tensor("pk", [pk_elems], F16, kind="ExternalInput").ap()
    if use_cc:
        outT = nc.dram_tensor("outT", [xsh, t], F16, kind="ExternalOutput").ap()
    else:
        outT = nc.dram_tensor("outT", [d_model, t], F16, kind="ExternalOutput").ap()

    def pk_ap(o, dims):
        return bass.AP(tensor=pk.tensor, offset=o,
                       ap=[[int(s), int(n)] for s, n in dims])

    def _body(tc):
        with ExitStack() as octx:
            consts = octx.enter_context(tc.tile_pool(name="consts", bufs=1))

            # fp16 staging + on-chip convert of the DVE-facing constants
            # (input-independent rope/mask tables — hoisted out of the
            # rep loop; everything input-dependent is inside emit_rep)
            stg_cos = consts.tile([128, t], F16, tag="stg_cos")
            nc.sync.dma_start(stg_cos[:], pk_ap(off["cos"], [(t, 128), (1, t)]))
            stg_sin = consts.tile([128, t], F16, tag="stg_sin")
            nc.sync.dma_start(stg_sin[:], pk_ap(off["sin"], [(t, 128), (1, t)]))
            stg_mask = consts.tile([128, 4, 512], F16, tag="stg_mask")
            nc.sync.dma_start(stg_mask[:],
                              pk_ap(off["mask"],
                                    [(512, 128), (128 * 512, 4), (1, 512)]))
            stg_psw = consts.tile([128, 128], F16, tag="stg_psw")
            nc.gpsimd.dma_start(stg_psw[:],
                                pk_ap(off["pswap"], [(128, 128), (1, 128)]))
            stg_id = consts.tile([128, 128], F16, tag="stg_id")
            nc.gpsimd.dma_start(stg_id[:],
                                pk_ap(off["ident"], [(128, 128), (1, 128)]))

            sb_cos = consts.tile([128, t], F32, tag="cos")
            nc.vector.tensor_copy(sb_cos[:], stg_cos[:])
            sb_sin = consts.tile([128, t], F32, tag="sin")
            nc.vector.tensor_copy(sb_sin[:], stg_sin[:])
            sb_mask = consts.tile([128, 4, 512], F32, tag="mask")
            nc.vector.tensor_copy(sb_mask[:], stg_mask[:])
            sb_pswap = consts.tile([128, 128], F32R, tag="pswap")
            nc.vector.tensor_copy(sb_pswap[:], stg_psw[:])
            sb_ident = consts.tile([128, 128], F32R, tag="ident")
            nc.vector.tensor_copy(sb_ident[:], stg_id[:])

            hsh = xsh // 2
            if use_cc:
                # gather buffers double-buffered across reps so rep r+1's
                # AllGather (issued during rep r's compute, BEFORE rep r's
                # ReduceScatter hits the in-order collective engine) can
                # land while rep r still reads its own gather.
                gdram = octx.enter_context(
                    tc.tile_pool(name="gdram", bufs=2, space="DRAM"))

            def emit_gather(rep):
                # all-gather the x shard in two halves so phase 1 can start
                # after the first half lands instead of the full 25MB.
                # AG-half h output rows: for each core c, its shard rows
                # [h*xsh/2, (h+1)*xsh/2) at output offset c*xsh/2.
                # Shared addr space: HBM-HBM collective fast path.
                xg = [gdram.tile([n_cores * hsh, t], F16, tag=f"xg{h}",
                                 name=f"xg{h}_r{rep}", addr_space="Shared")
                      for h in range(2)]
                if "ag" not in _skip:
                    xg_in0 = gdram.tile([hsh, t], F16, tag="xg_in0",
                                        name=f"xg_in0_r{rep}")
                    nc.gpsimd.dma_start(
                        xg_in0[:], pk_ap(off["xs"], [(t, hsh), (1, t)]))
                    xg_in1 = gdram.tile([hsh, t], F16, tag="xg_in1",
                                        name=f"xg_in1_r{rep}")
                    nc.gpsimd.dma_start(
                        xg_in1[:], pk_ap(off["xs"] + hsh * t,
                                         [(t, hsh), (1, t)]))
                    nc.gpsimd.collective_compute(
                        "AllGather", mybir.AluOpType.bypass,
                        replica_groups=[list(range(n_cores))],
                        ins=[xg_in0.opt()], outs=[xg[0].opt()])
                    nc.gpsimd.collective_compute(
                        "AllGather", mybir.AluOpType.bypass,
                        replica_groups=[list(range(n_cores))],
                        ins=[xg_in1.opt()], outs=[xg[1].opt()])
                else:
                    # timing-only variant: fill locally, no NeuronLink
                    for h in range(2):
                        nc.gpsimd.dma_start(
                            xg[h][:hsh, :],
                            pk_ap(off["xs"] + h * hsh * t,
                                  [(t, hsh), (1, t)]))
                return xg

            def emit_rep(rep, xg, next_gather):
                with ExitStack() as rctx:
                    dram = rctx.enter_context(
                        tc.tile_pool(name=f"dram_r{rep}", bufs=1, space="DRAM"))
                    if use_cc:
                        def x_tile_src(k):
                            g = k * 128              # global xT row
                            c, r = g // xsh, g % xsh
                            h, rr = r // hsh, r % hsh
                            row = ((c * hsh + rr) if "ag" not in _skip
                                   else rr % hsh)
                            return xg[h][row:row + 128, :]
                    else:
                        def x_tile_src(k):
                            return pk_ap(off["xs"] + k * 128 * t,
                                         [(t, 128), (1, t)])

                    accp = rctx.enter_context(
                        tc.tile_pool(name=f"accp_r{rep}", bufs=nf))
                    acc = [accp.tile([128, t], F32R, tag="acc",
                                     name=f"acc{i}_r{rep}")
                           for i in range(nf)]

                    # ---------- Phase 1: qkvT = wqkvT-tiles.T @ xT ----------
                    # fp16 matmuls; weight-stationary: one w block feeds 4
                    # PSUM banks.
                    with ExitStack() as ctx:
                        xp = ctx.enter_context(tc.tile_pool(name="xp", bufs=10))
                        wp = ctx.enter_context(tc.tile_pool(name="wp", bufs=9))
                        psp = ctx.enter_context(
                            tc.tile_pool(name="psp", bufs=2 * tq, space="PSUM"))
                        plan = _chunk_plan(kd, chunk)
                        if use_cc:
                            # AG-half-0 k-tiles first so phase 1 starts at
                            # half-gather
                            nh = xsh // 256          # k-tiles per core half
                            k_order = ([k for k in range(kd)
                                        if k % (2 * nh) < nh]
                                       + [k for k in range(kd)
                                          if k % (2 * nh) >= nh])
                        else:
                            k_order = list(range(kd))
                        base = 0
                        for c, csz in enumerate(plan):
                            ks = k_order[base:base + csz]
                            base += csz
                            xts, wts = [], []
                            for k in ks:
                                xt = xp.tile([128, t], F16, tag="xt")
                                nc.sync.dma_start(xt[:], x_tile_src(k))
                                xts.append(xt)
                                wt = wp.tile([128, qkv_cols], F16, tag="wt")
                                nc.sync.dma_start(
                                    wt[:], pk_ap(off["wq"] + k * 128 * qkv_cols,
                                                 [(qkv_cols, 128),
                                                  (1, qkv_cols)]))
                                wts.append(wt)
                            for f in range(nf):
                                pss = [psp.tile([128, 512], F32, tag="ps",
                                                name=f"ps{q}")
                                       for q in range(tq)]
                                for i in range(len(ks)):
                                    lhs = wts[i][:, f * 128:(f + 1) * 128]
                                    for q in range(tq):
                                        nc.tensor.matmul(
                                            pss[q][:], lhs,
                                            xts[i][:, q * 512:(q + 1) * 512],
                                            start=(i == 0),
                                            stop=(i == len(ks) - 1),
                                        )
                                for q in range(tq):
                                    sl = slice(q * 512, (q + 1) * 512)
                                    if c == 0:
                                        nc.vector.tensor_copy(acc[f][:, sl],
                                                              pss[q][:])
                                    else:
                                        nc.vector.scalar_tensor_tensor(
                                            out=acc[f][:, sl], in0=pss[q][:],
                                            scalar=1.0, in1=acc[f][:, sl],
                                            op0=MULT, op1=ADD,
                                        )

                    # issue the NEXT rep's gather now: its input DMAs queue
                    # behind phase 1's tile loads, and its AllGather reaches
                    # the in-order collective engine BEFORE this rep's
                    # ReduceScatter — so phase 1 of rep+1 never waits on a
                    # collective, and the RS hides under rep+1's compute.
                    if next_gather is not None:
                        next_gather()

                    # late pools: open only after phase 1 frees its SBUF
                    attp = rctx.enter_context(
                        tc.tile_pool(name=f"attp_r{rep}", bufs=nqh))
                    vtp = rctx.enter_context(
                        tc.tile_pool(name=f"vtp_r{rep}", bufs=1))
                    attnT = [attp.tile([128, t], F16, tag="attnT",
                                       name=f"attnT{i}_r{rep}")
                             for i in range(nqh)]
                    v_t = vtp.tile([128, tb, 128], F32R, tag="v_t")

                    # ------ Phase 1.5: RoPE + clamp + v transpose ------
                    # Emission order matters (DVE is FIFO): k head first,
                    # then v work, then q heads in pair order — the attention
                    # loop below is hp-outer, so rope for later head pairs
                    # hides under the PE work of earlier pairs.
                    rope_ctx = ExitStack()
                    # shared PSUM pool: rope pair-swap tiles + attention
                    # score tiles
                    spp = rope_ctx.enter_context(
                        tc.tile_pool(name="spp", bufs=6, space="PSUM"))
                    tmp = rope_ctx.enter_context(
                        tc.tile_pool(name="rtmp", bufs=3))

                    def emit_rope(f):
                        for q in range(tq):
                            sl = slice(q * 512, (q + 1) * 512)
                            pshuf = spp.tile([128, 512], F32, tag="ps",
                                             name="pshuf")
                            nc.tensor.matmul(pshuf[:], sb_pswap[:],
                                             acc[f][:, sl],
                                             start=True, stop=True)
                            t1 = tmp.tile([128, 512], F32, tag="t1", name="t1")
                            nc.vector.tensor_tensor(t1[:],
                                                    acc[f][:, sl].bitcast(F32),
                                                    sb_cos[:, sl], op=MULT)
                            t2 = tmp.tile([128, 512], F32, tag="t2", name="t2")
                            nc.vector.tensor_tensor(t2[:], pshuf[:],
                                                    sb_sin[:, sl], op=MULT)
                            t3 = tmp.tile([128, 512], F32, tag="t3", name="t3")
                            nc.vector.tensor_tensor(t3[:], t1[:], t2[:], op=ADD)
                            nc.vector.tensor_scalar(acc[f][:, sl], t3[:],
                                                    -CLAMP, CLAMP,
                                                    op0=MAX, op1=MIN)

                    def emit_vwork():
                        iv = nqh + 1          # v: clamp only, then transpose
                        for q in range(tq):
                            sl = slice(q * 512, (q + 1) * 512)
                            t4 = tmp.tile([128, 512], F32, tag="t1", name="t1")
                            nc.vector.tensor_scalar(t4[:],
                                                    acc[iv][:, sl].bitcast(F32),
                                                    -CLAMP, CLAMP,
                                                    op0=MAX, op1=MIN)
                            nc.vector.tensor_copy(acc[iv][:, sl], t4[:])
                        for j in range(tb):
                            pt = spp.tile([128, 128], F32R, tag="ps", name="pt")
                            nc.tensor.transpose(pt[:],
                                                acc[iv][:, j * 128:(j + 1) * 128],
                                                sb_ident[:])
                            nc.vector.tensor_copy(v_t[:, j, :], pt[:])

                    emit_rope(nqh)                        # k head
                    emit_vwork()

                    # ------ Phase 2: causal attention (head pairs) ------
                    # Software-pipelined: score matmuls + exp run PIPE
                    # kb-steps ahead of the PV matmuls so the PE never waits
                    # on the ACT round-trip. The softmax denominator runs on
                    # the (otherwise idle) Pool engine: exp tiles accumulate
                    # elementwise, then one cross-partition all-reduce per
                    # (head, quad) — no PE ones-matmuls, and the freed PSUM
                    # banks deepen the score pipeline. hp-outer so
                    # emit_rope(f) for pair p+1 overlaps pair p's matmuls.
                    PIPE = 3
                    with ExitStack() as ctx:
                        exps = ctx.enter_context(
                            tc.tile_pool(name="exps", bufs=2 * (PIPE + 1)))
                        psa_p = ctx.enter_context(
                            tc.tile_pool(name="psa", bufs=2, space="PSUM"))
                        esp = ctx.enter_context(tc.tile_pool(name="esp", bufs=2))
                        rcb = ctx.enter_context(tc.tile_pool(name="rcb", bufs=2))
                        ik = nqh                  # k head feature tile
                        for hp in range(nqh // 2):
                            hs = (2 * hp, 2 * hp + 1)
                            emit_rope(hs[0])
                            emit_rope(hs[1])
                            for J in range(tq):
                                qsl = slice(J * 512, (J + 1) * 512)
                                nkb = 4 * J + 4
                                pa = {h: psa_p.tile([128, 512], F32, tag="pa",
                                                    name="pa")
                                      for h in hs}
                                es = {h: esp.tile([128, 512], F32R, tag="es",
                                                  name="es")
                                      for h in hs}
                                exq = {}
                                for kb in range(nkb + PIPE):
                                    if kb < nkb:
                                        klhs = acc[ik][:, kb * 128:(kb + 1) * 128]
                                        for h in hs:
                                            ps = spp.tile([128, 512], F32,
                                                          tag="ps", name="ps")
                                            nc.tensor.matmul(
                                                ps[:], klhs, acc[h][:, qsl],
                                                start=True, stop=True)
                                            if kb >= 4 * J:
                                                nc.vector.tensor_tensor(
                                                    ps[:], ps[:],
                                                    sb_mask[:, kb - 4 * J, :],
                                                    op=ADD)
                                            ex = exps.tile([128, 512], F32R,
                                                           tag="ex", name="ex")
                                            nc.scalar.activation(ex[:], ps[:],
                                                                 EXP,
                                                                 scale=SCALE)
                                            exq[(kb, h)] = ex
                                    kbd = kb - PIPE
                                    if kbd < 0 or kbd >= nkb:
                                        continue
                                    st = (kbd == 0)
                                    sp = (kbd == nkb - 1)
                                    for h in hs:
                                        nc.tensor.matmul(pa[h][:],
                                                         v_t[:, kbd, :],
                                                         exq[(kbd, h)][:],
                                                         start=st, stop=sp)
                                    for h in hs:
                                        ex = exq.pop((kbd, h))
                                        if st:
                                            nc.vector.tensor_copy(es[h][:],
                                                                  ex[:])
                                        else:
                                            nc.vector.tensor_tensor(
                                                es[h][:], es[h][:], ex[:],
                                                op=ADD)
                                for h in hs:
                                    # cross-partition sum -> broadcast, then
                                    # reciprocal; result is already [128, 512]
                                    # so no partition-broadcast DMA is needed
                                    pr = esp.tile([128, 512], F32R, tag="pr",
                                                  name="pr")
                                    nc.gpsimd.partition_all_reduce(
                                        pr[:], es[h][:], channels=128,
                                        reduce_op=bass_isa.ReduceOp.add)
                                    rb = rcb.tile([128, 512], F32R, tag="rb",
                                                  name="rb")
                                    nc.vector.reciprocal(rb[:], pr[:])
                                    nc.vector.tensor_tensor(attnT[h][:, qsl],
                                                            pa[h][:],
                                                            rb[:], op=MULT)

                    rope_ctx.close()

                    # ------ Phase 3: partial out projection (fp16) ------
                    # Weight-stationary: one w_o block feeds all 4 t-quads.
                    # Partials land in a DRAM buffer, then on-device
                    # ReduceScatter (Shared output) sums them and leaves this
                    # core's [xsh, t] shard for the tiny output DMA.
                    if use_cc:
                        # two half-partials: RS of the first half overlaps the
                        # PE work of the second half of the o-loop. Core c's
                        # output shard is [global rows 384c..384c+384] ++
                        # [3072+384c..3072+384c+384]; the host undoes this.
                        hd2 = d_model // 2
                        part = [dram.tile([hd2, t], F16, tag=f"part{h}",
                                          name=f"part{h}_r{rep}")
                                for h in range(2)]
                        rs_out = [dram.tile([xsh // 2, t], F16, tag=f"rs{h}",
                                            name=f"rs{h}_r{rep}")
                                  for h in range(2)]
                    with ExitStack() as ctx:
                        wop = ctx.enter_context(tc.tile_pool(name="wop",
                                                             bufs=4))
                        outp = ctx.enter_context(tc.tile_pool(name="outp",
                                                              bufs=2 * tq))
                        pso = ctx.enter_context(
                            tc.tile_pool(name="pso", bufs=2 * tq, space="PSUM"))
                        for o in range(d_model // 128):
                            wo = wop.tile([128, nqh, 128], F16, tag="wo")
                            # w_o[s*128+p, o*128+oo] lives at flat off["wo"] +
                            # 6144*(s*128+p) + 128*o + oo
                            nc.sync.dma_start(
                                wo[:], pk_ap(off["wo"] + o * 128,
                                             [(d_model, 128),
                                              (d_model * 128, nqh),
                                              (1, 128)]))
                            pos = [pso.tile([128, 512], F32, tag="po",
                                            name=f"po{q}")
                                   for q in range(tq)]
                            for s in range(nqh):
                                lhs = wo[:, s, :]
                                for J in range(tq):
                                    nc.tensor.matmul(
                                        pos[J][:], lhs,
                                        attnT[s][:, J * 512:(J + 1) * 512],
                                        start=(s == 0), stop=(s == nqh - 1))
                            for J in range(tq):
                                ob = outp.tile([128, 512], F16, tag="ob",
                                               name="ob")
                                nc.vector.tensor_copy(ob[:], pos[J][:])
                                if use_cc:
                                    dst = (part[0] if o * 128 < d_model // 2
                                           else part[1])
                                    ro = o * 128 - (0 if o * 128 < d_model // 2
                                                    else d_model // 2)
                                else:
                                    dst, ro = outT, o * 128
                                nc.sync.dma_start(
                                    dst[ro:ro + 128, J * 512:(J + 1) * 512],
                                    ob[:])

                    if use_cc:
                        for h in range(2):
                            if "rs" not in _skip:
                                nc.gpsimd.collective_compute(
                                    "ReduceScatter", ADD,
                                    replica_groups=[list(range(n_cores))],
                                    ins=[part[h].opt()], outs=[rs_out[h].opt()])
                                nc.sync.dma_start(
                                    outT[h * (xsh // 2):(h + 1) * (xsh // 2), :],
                                    rs_out[h][:])
                            else:
                                # timing-only: skip the collective, emit the
                                # same-size final DMA from the partial buffer
                                nc.sync.dma_start(
                                    outT[h * (xsh // 2):(h + 1) * (xsh // 2), :],
                                    part[h][:xsh // 2, :])

            if use_cc:
                gathered = {0: emit_gather(0)}

                def make_next(r):
                    def cb():
                        gathered[r] = emit_gather(r)
                    return cb

                for rep in range(reps):
                    emit_rep(rep, gathered.pop(rep),
                             make_next(rep + 1) if rep + 1 < reps else None)
            else:
                for rep in range(reps):
                    emit_rep(rep, None, None)

    with tile.TileContext(nc) as tc, nc.allow_low_precision(reason="fp16/fp32r matmuls"):
        _body(tc)
    nc.compile()
    return nc


def make_core_inputs(x, causal_mask, w_qkv, w_out, d_model=D_MODEL, t=T, nqh=NQH,
                     n_cores=N_CORES, use_cc=True):
    """Host-side sharding: per-core input dicts for the SPMD program."""
    x2 = np.ascontiguousarray(x.reshape(t, d_model).T).astype(np.float16)  # [D, T]
    kv_base = nqh * n_cores * HEAD_DIM
    n_kv = n_cores
    xsh = d_model // n_cores

    # RoPE tables in [head_dim, T] layout (interleaved-pair convention).
    hd = HEAD_DIM
    inv = 1.0 / ROPE_BASE ** (np.arange(0, hd, 2, dtype=np.float64) / hd)
    pos = np.arange(t, dtype=np.float64)
    freqs = pos[None, :] * inv[:, None]                     # [hd/2, T]
    cos = np.cos(freqs)
    sin = np.sin(freqs)
    cosf = np.empty((hd, t), dtype=np.float16)
    sinf = np.empty((hd, t), dtype=np.float16)
    cosf[0::2] = cos
    cosf[1::2] = cos
    sinf[0::2] = -sin                                       # row 2i:   -sin
    sinf[1::2] = sin                                        # row 2i+1: +sin

    # causal mask diagonal-quad slices, clipped to an fp16-safe big-negative
    cm = causal_mask.reshape(causal_mask.shape[-2], causal_mask.shape[-1])
    cm = np.maximum(np.asarray(cm, dtype=np.float32), MASK_NEG)
    maskT = np.stack([np.ascontiguousarray(cm[0:512, m * 128:(m + 1) * 128].T)
                      for m in range(4)]).astype(np.float16)

    pswap = np.zeros((128, 128), dtype=np.float16)
    for i in range(0, 128, 2):
        pswap[i, i + 1] = 1.0
        pswap[i + 1, i] = 1.0
    ident = np.eye(128, dtype=np.float16)
    ones = np.ones(128, dtype=np.float16)

    in_maps = []
    for c in range(n_cores):
        qrows = np.arange(c * nqh * 128, (c + 1) * nqh * 128)
        krows = np.arange(kv_base + c * 128, kv_base + (c + 1) * 128)
        vrows = np.arange(kv_base + n_kv * 128 + c * 128,
                          kv_base + n_kv * 128 + (c + 1) * 128)
        rows = np.concatenate([qrows, krows, vrows])
        wqkvT_c = np.ascontiguousarray(w_qkv[rows, :].T).astype(np.float16)
        w_o_c = np.ascontiguousarray(w_out[:, qrows].T).astype(np.float16)
        xs_c = (np.ascontiguousarray(x2[c * xsh:(c + 1) * xsh, :])
                if use_cc else x2)
        pk = np.concatenate([xs_c.ravel(), wqkvT_c.ravel(), w_o_c.ravel(),
                             cosf.ravel(), sinf.ravel(), maskT.ravel(),
                             pswap.ravel(), ident.ravel(), ones])
        in_maps.append({"pk": pk})
    return in_maps


_PROGRAM_CACHE = {}


def _get_program(reps=1):
    key = (D_MODEL, T, NQH, reps)
    if key not in _PROGRAM_CACHE:
        _PROGRAM_CACHE[key] = build_program(reps=reps)
    return _PROGRAM_CACHE[key]


def kernel(x, causal_mask, w_qkv, w_out):
    x = np.asarray(x, dtype=np.float32)
    causal_mask = np.asarray(causal_mask, dtype=np.float32)
    w_qkv = np.asarray(w_qkv, dtype=np.float32)
    w_out = np.asarray(w_out, dtype=np.float32)

    nc = _get_program()
    in_maps = make_core_inputs(x, causal_mask, w_qkv, w_out)
    res = run_bass_kernel_spmd(nc, in_maps, list(range(N_CORES)))
    shards = [np.asarray(res.results[c]["outT"], dtype=np.float32)
              for c in range(N_CORES)]
    h = XSH // 2
    top = np.concatenate([s[:h] for s in shards], axis=0)   # rows 0..D/2
    bot = np.concatenate([s[h:] for s in shards], axis=0)   # rows D/2..D
    outT = np.concatenate([top, bot], axis=0)               # [D, T]
    return np.ascontiguousarray(outT.T).reshape(1, T, D_MODEL).astype(np.float32)


# revision 24
# speedup vs baseline: 1.0445x; 1.0445x over previous
"""DBRX attention block (B=1, T=2048, D=6144, 48 q heads / 8 kv heads, RoPE,
clamp, causal) as a Bass/Tile kernel on 8 Trainium2 NeuronCores.

Sharding: tensor-parallel over heads. Core c owns q heads [6c, 6c+6) and kv
head c (GQA groups align exactly: q head i uses kv head i//6).

Pipeline per core (feature-major [feature, T] layout so every matmul is a
clean PE op at full rate; fp16 projections — fp8 was measured at ~5e-2
rel err, over the gate):
  - x ships SHARDED (768 rows of xT per core, fp16) and is all-gathered
    on-device over NeuronLink into a Shared-address-space HBM buffer,
  - qkvT = wqkvT.T @ xT in fp16 (PSUM accumulates fp32),
  - RoPE via a pair-swap permutation matmul + DVE combine (fp32r),
  - scoresT[k, q] = kT.T @ qT (fp32r, full rate at 512-wide); exp on ACT
    (no max-subtraction: scores bounded by the clamp, fp32 range holds),
  - attnT = v @ expT (fp32r); softmax denominator on the Pool engine
    (elementwise exp-tile accumulation + one cross-partition all-reduce
    per head/quad), applied with a DVE reciprocal,
  - outT = w_o.T @ attnT (fp16), partials ReduceScattered (Shared output)
    so each core ships back only its [768, 2048] fp16 shard.

reps>1 unrolls the whole body (incl. collectives) N times back-to-back in
one program for steady-state marginal-cost timing; see build_program.
"""

import math
import sys
from contextlib import ExitStack

import numpy as np

for _p in ("/opt/trn_rl_repo", "/root/.axon_site/_ro/trn_rl_repo"):
    if _p not in sys.path:
        sys.path.append(_p)

import concourse.bass as bass
import concourse.tile as tile
from concourse import bacc, bass_isa, mybir
from concourse.bass_utils import run_bass_kernel_spmd

F16 = mybir.dt.float16
F32 = mybir.dt.float32
F32R = mybir.dt.float32r
MULT = mybir.AluOpType.mult
ADD = mybir.AluOpType.add
MAX = mybir.AluOpType.max
MIN = mybir.AluOpType.min
EXP = mybir.ActivationFunctionType.Exp

N_CORES = 8
D_MODEL = 6144
N_HEADS = 48
N_KV_HEADS = 8
HEAD_DIM = 128
NQH = N_HEADS // N_CORES          # q heads per core = 6
T = 2048
CLAMP = 8.0
ROPE_BASE = 500000.0
SCALE = 1.0 / math.sqrt(HEAD_DIM)
MASK_NEG = -30000.0               # fp16-safe; SCALE*MASK_NEG << -90 => exp = 0
XSH = D_MODEL // N_CORES          # x shard rows per core = 768


def _chunk_plan(kd, chunk):
    """Chunk sizes: small leading chunks so the PE starts early."""
    plan = []
    rest = kd
    if kd > 8:
        plan = [1, 2, 3]
        rest = kd - 6
    while rest > 0:
        c = min(chunk, rest)
        plan.append(c)
        rest -= c
    return plan


def _pk_offsets(d_model, t, nqh, n_cores, use_cc):
    """Element offsets of each logical tensor inside the flat fp16 operand."""
    xsh = d_model // n_cores
    nf = nqh + 2
    qkv_cols = nf * 128
    off = {}
    cur = 0
    off["xs"] = cur
    cur += (xsh if use_cc else d_model) * t
    off["wq"] = cur
    cur += d_model * qkv_cols
    off["wo"] = cur
    cur += nqh * 128 * d_model
    off["cos"] = cur
    cur += 128 * t
    off["sin"] = cur
    cur += 128 * t
    off["mask"] = cur
    cur += 4 * 128 * 512
    off["pswap"] = cur
    cur += 128 * 128
    off["ident"] = cur
    cur += 128 * 128
    off["ones"] = cur
    cur += 128
    return off, cur


def build_program(d_model=D_MODEL, t=T, nqh=NQH, chunk=6, n_cores=N_CORES,
                  use_cc=True, reps=1, _skip=()):
    """Build the per-core Bass program. Returns the compiled Bacc handle.

    use_cc=False builds a single-core variant (full xT input, full partial
    output, no collectives) for CoreSim numerical checking.

    reps>1 unrolls the whole kernel body (phases 1-3 including the
    collectives) N times back-to-back inside one program execution, for
    steady-state device timing: per-invocation time is the marginal cost
    of one extra rep. Each rep re-reads x and all weights from DRAM and
    re-runs the AllGather/ReduceScatter, so one rep is exactly one honest
    kernel invocation; only the tiny input-independent constant staging
    (rope/mask tables, ~1.3 MB) is hoisted out of the loop.
    """
    kd = d_model // 128            # contraction tiles for qkv projection
    tq = t // 512                  # 512-wide T quads
    tb = t // 128                  # 128-wide T blocks
    nf = nqh + 2                   # feature tiles: q heads + k + v
    qkv_cols = nf * 128
    xsh = d_model // n_cores       # x shard rows

    off, pk_elems = _pk_offsets(d_model, t, nqh, n_cores, use_cc)

    nc = bacc.Bacc("TRN2", target_bir_lowering=False, debug=False,
                   num_devices=n_cores)

    pk = nc.dram_tensor("pk", [pk_elems], F16, kind="ExternalInput").ap()
    if use_cc:
        outT = nc.dram_tensor("outT", [xsh, t], F16, kind="ExternalOutput").ap()
    else:
        outT = nc.dram_tensor("outT", [d_model, t], F16, kind="ExternalOutput").ap()

    def pk_ap(o, dims):
        return bass.AP(tensor=pk.tensor, offset=o,
                       ap=[[int(s), int(n)] for s, n in dims])

    def _body(tc):
        with ExitStack() as octx:
            consts = octx.enter_context(tc.tile_pool(name="consts", bufs=1))

            # fp16 staging + on-chip convert of the DVE-facing constants
            # (input-independent rope/mask tables — hoisted out of the
            # rep loop; everything input-dependent is inside emit_rep)
            stg_cos = consts.tile([128, t], F16, tag="stg_cos")
            nc.sync.dma_start(stg_cos[:], pk_ap(off["cos"], [(t, 128), (1, t)]))
            stg_sin = consts.tile([128, t], F16, tag="stg_sin")
            nc.sync.dma_start(stg_sin[:], pk_ap(off["sin"], [(t, 128), (1, t)]))
            stg_mask = consts.tile([128, 4, 512], F16, tag="stg_mask")
            nc.sync.dma_start(stg_mask[:],
                              pk_ap(off["mask"],
                                    [(512, 128), (128 * 512, 4), (1, 512)]))
            stg_psw = consts.tile([128, 128], F16, tag="stg_psw")
            nc.gpsimd.dma_start(stg_psw[:],
                                pk_ap(off["pswap"], [(128, 128), (1, 128)]))
            stg_id = consts.tile([128, 128], F16, tag="stg_id")
            nc.gpsimd.dma_start(stg_id[:],
                                pk_ap(off["ident"], [(128, 128), (1, 128)]))

            sb_cos = consts.tile([128, t], F32, tag="cos")
            nc.vector.tensor_copy(sb_cos[:], stg_cos[:])
            sb_sin = consts.tile([128, t], F32, tag="sin")
            nc.vector.tensor_copy(sb_sin[:], stg_sin[:])
            sb_mask = consts.tile([128, 4, 512], F32, tag="mask")
            nc.vector.tensor_copy(sb_mask[:], stg_mask[:])
            sb_pswap = consts.tile([128, 128], F32R, tag="pswap")
            nc.vector.tensor_copy(sb_pswap[:], stg_psw[:])
            sb_ident = consts.tile([128, 128], F32R, tag="ident")
            nc.vector.tensor_copy(sb_ident[:], stg_id[:])

            hsh = xsh // 2
            if use_cc:
                # gather buffers double-buffered across reps so rep r+1's
                # AllGather (issued during rep r's compute, BEFORE rep r's
                # ReduceScatter hits the in-order collective engine) can
                # land while rep r still reads its own gather.
                gdram = octx.enter_context(
                    tc.tile_pool(name="gdram", bufs=2, space="DRAM"))

            def emit_gather(rep):
                # all-gather the x shard in two halves so phase 1 can start
                # after the first half lands instead of the full 25MB.
                # AG-half h output rows: for each core c, its shard rows
                # [h*xsh/2, (h+1)*xsh/2) at output offset c*xsh/2.
                # Shared addr space: HBM-HBM collective fast path.
                xg = [gdram.tile([n_cores * hsh, t], F16, tag=f"xg{h}",
                                 name=f"xg{h}_r{rep}", addr_space="Shared")
                      for h in range(2)]
                if "ag" not in _skip:
                    xg_in0 = gdram.tile([hsh, t], F16, tag="xg_in0",
                                        name=f"xg_in0_r{rep}")
                    nc.gpsimd.dma_start(
                        xg_in0[:], pk_ap(off["xs"], [(t, hsh), (1, t)]))
                    xg_in1 = gdram.tile([hsh, t], F16, tag="xg_in1",
                                        name=f"xg_in1_r{rep}")
                    nc.gpsimd.dma_start(
                        xg_in1[:], pk_ap(off["xs"] + hsh * t,
                                         [(t, hsh), (1, t)]))
                    nc.gpsimd.collective_compute(
                        "AllGather", mybir.AluOpType.bypass,
                        replica_groups=[list(range(n_cores))],
                        ins=[xg_in0.opt()], outs=[xg[0].opt()])
                    nc.gpsimd.collective_compute(
                        "AllGather", mybir.AluOpType.bypass,
                        replica_groups=[list(range(n_cores))],
                        ins=[xg_in1.opt()], outs=[xg[1].opt()])
                else:
                    # timing-only variant: fill locally, no NeuronLink
                    for h in range(2):
                        nc.gpsimd.dma_start(
                            xg[h][:hsh, :],
                            pk_ap(off["xs"] + h * hsh * t,
                                  [(t, hsh), (1, t)]))
                return xg

            def emit_rep(rep, xg, next_gather):
                with ExitStack() as rctx:
                    dram = rctx.enter_context(
                        tc.tile_pool(name=f"dram_r{rep}", bufs=1, space="DRAM"))
                    if use_cc:
                        def x_tile_src(k):
                            g = k * 128              # global xT row
                            c, r = g // xsh, g % xsh
                            h, rr = r // hsh, r % hsh
                            row = ((c * hsh + rr) if "ag" not in _skip
                                   else rr % hsh)
                            return xg[h][row:row + 128, :]
                    else:
                        def x_tile_src(k):
                            return pk_ap(off["xs"] + k * 128 * t,
                                         [(t, 128), (1, t)])

                    accp = rctx.enter_context(
                        tc.tile_pool(name=f"accp_r{rep}", bufs=nf))
                    acc = [accp.tile([128, t], F32R, tag="acc",
                                     name=f"acc{i}_r{rep}")
                           for i in range(nf)]

                    # ---------- Phase 1: qkvT = wqkvT-tiles.T @ xT ----------
                    # fp16 matmuls; weight-stationary: one w block feeds 4
                    # PSUM banks.
                    with ExitStack() as ctx:
                        xp = ctx.enter_context(tc.tile_pool(name="xp", bufs=10))
                        wp = ctx.enter_context(tc.tile_pool(name="wp", bufs=9))
                        psp = ctx.enter_context(
                            tc.tile_pool(name="psp", bufs=2 * tq, space="PSUM"))
                        plan = _chunk_plan(kd, chunk)
                        if use_cc:
                            # AG-half-0 k-tiles first so phase 1 starts at
                            # half-gather
                            nh = xsh // 256          # k-tiles per core half
                            k_order = ([k for k in range(kd)
                                        if k % (2 * nh) < nh]
                                       + [k for k in range(kd)
                                          if k % (2 * nh) >= nh])
                        else:
                            k_order = list(range(kd))
                        base = 0
                        for c, csz in enumerate(plan):
                            ks = k_order[base:base + csz]
                            base += csz
                            xts, wts = [], []
                            for k in ks:
                                xt = xp.tile([128, t], F16, tag="xt")
                                nc.sync.dma_start(xt[:], x_tile_src(k))
                                xts.append(xt)
                                wt = wp.tile([128, qkv_cols], F16, tag="wt")
                                nc.sync.dma_start(
                                    wt[:], pk_ap(off["wq"] + k * 128 * qkv_cols,
                                                 [(qkv_cols, 128),
                                                  (1, qkv_cols)]))
                                wts.append(wt)
                            for f in range(nf):
                                pss = [psp.tile([128, 512], F32, tag="ps",
                                                name=f"ps{q}")
                                       for q in range(tq)]
                                for i in range(len(ks)):
                                    lhs = wts[i][:, f * 128:(f + 1) * 128]
                                    for q in range(tq):
                                        nc.tensor.matmul(
                                            pss[q][:], lhs,
                                            xts[i][:, q * 512:(q + 1) * 512],
                                            start=(i == 0),
                                            stop=(i == len(ks) - 1),
                                        )
                                for q in range(tq):
                                    sl = slice(q * 512, (q + 1) * 512)
                                    if c == 0:
                                        nc.vector.tensor_copy(acc[f][:, sl],
                                                              pss[q][:])
                                    else:
                                        nc.vector.scalar_tensor_tensor(
                                            out=acc[f][:, sl], in0=pss[q][:],
                                            scalar=1.0, in1=acc[f][:, sl],
                                            op0=MULT, op1=ADD,
                                        )

                    # issue the NEXT rep's gather now: its input DMAs queue
                    # behind phase 1's tile loads, and its AllGather reaches
                    # the in-order collective engine BEFORE this rep's
                    # ReduceScatter — so phase 1 of rep+1 never waits on a
                    # collective, and the RS hides under rep+1's compute.
                    if next_gather is not None:
                        next_gather()

                    # late pools: open only after phase 1 frees its SBUF
                    attp = rctx.enter_context(
                        tc.tile_pool(name=f"attp_r{rep}", bufs=nqh))
                    vtp = rctx.enter_context(
                        tc.tile_pool(name=f"vtp_r{rep}", bufs=1))
                    attnT = [attp.tile([128, t], F16, tag="attnT",
                                       name=f"attnT{i}_r{rep}")
                             for i in range(nqh)]
                    v_t = vtp.tile([128, tb, 128], F32R, tag="v_t")

                    # ------ Phase 1.5: RoPE + clamp + v transpose ------
                    # Emission order matters (DVE is FIFO): k head first,
                    # then v work, then q heads in pair order — the attention
                    # loop below is hp-outer, so rope for later head pairs
                    # hides under the PE work of earlier pairs.
                    rope_ctx = ExitStack()
                    # shared PSUM pool: rope pair-swap tiles + attention
                    # score tiles
                    spp = rope_ctx.enter_context(
                        tc.tile_pool(name="spp", bufs=6, space="PSUM"))
                    tmp = rope_ctx.enter_context(
                        tc.tile_pool(name="rtmp", bufs=3))

                    def emit_rope(f):
                        for q in range(tq):
                            sl = slice(q * 512, (q + 1) * 512)
                            pshuf = spp.tile([128, 512], F32, tag="ps",
                                             name="pshuf")
                            nc.tensor.matmul(pshuf[:], sb_pswap[:],
                                             acc[f][:, sl],
                                             start=True, stop=True)
                            t1 = tmp.tile([128, 512], F32, tag="t1", name="t1")
                            nc.vector.tensor_tensor(t1[:],
                                                    acc[f][:, sl].bitcast(F32),
                                                    sb_cos[:, sl], op=MULT)
                            t2 = tmp.tile([128, 512], F32, tag="t2", name="t2")
                            nc.vector.tensor_tensor(t2[:], pshuf[:],
                                                    sb_sin[:, sl], op=MULT)
                            t3 = tmp.tile([128, 512], F32, tag="t3", name="t3")
                            nc.vector.tensor_tensor(t3[:], t1[:], t2[:], op=ADD)
                            nc.vector.tensor_scalar(acc[f][:, sl], t3[:],
                                                    -CLAMP, CLAMP,
                                                    op0=MAX, op1=MIN)

                    def emit_vwork():
                        iv = nqh + 1          # v: clamp only, then transpose
                        for q in range(tq):
                            sl = slice(q * 512, (q + 1) * 512)
                            t4 = tmp.tile([128, 512], F32, tag="t1", name="t1")
                            nc.vector.tensor_scalar(t4[:],
                                                    acc[iv][:, sl].bitcast(F32),
                                                    -CLAMP, CLAMP,
                                                    op0=MAX, op1=MIN)
                            nc.vector.tensor_copy(acc[iv][:, sl], t4[:])
                        for j in range(tb):
                            pt = spp.tile([128, 128], F32R, tag="ps", name="pt")
                            nc.tensor.transpose(pt[:],
                                                acc[iv][:, j * 128:(j + 1) * 128],
                                                sb_ident[:])
                            nc.vector.tensor_copy(v_t[:, j, :], pt[:])

                    emit_rope(nqh)                        # k head
                    emit_vwork()

                    # ------ Phase 2: causal attention (head pairs) ------
                    # Software-pipelined: score matmuls + exp run PIPE
                    # kb-steps ahead of the PV matmuls so the PE never waits
                    # on the ACT round-trip. The softmax denominator runs on
                    # the (otherwise idle) Pool engine: exp tiles accumulate
                    # elementwise, then one cross-partition all-reduce per
                    # (head, quad) — no PE ones-matmuls, and the freed PSUM
                    # banks deepen the score pipeline. hp-outer so
                    # emit_rope(f) for pair p+1 overlaps pair p's matmuls.
                    PIPE = 3
                    with ExitStack() as ctx:
                        exps = ctx.enter_context(
                            tc.tile_pool(name="exps", bufs=2 * (PIPE + 1)))
                        psa_p = ctx.enter_context(
                            tc.tile_pool(name="psa", bufs=2, space="PSUM"))
                        esp = ctx.enter_context(tc.tile_pool(name="esp", bufs=2))
                        rcb = ctx.enter_context(tc.tile_pool(name="rcb", bufs=2))
                        ik = nqh                  # k head feature tile
                        for hp in range(nqh // 2):
                            hs = (2 * hp, 2 * hp + 1)
                            emit_rope(hs[0])
                            emit_rope(hs[1])
                            for J in range(tq):
                                qsl = slice(J * 512, (J + 1) * 512)
                                nkb = 4 * J + 4
                                pa = {h: psa_p.tile([128, 512], F32, tag="pa",
                                                    name="pa")
                                      for h in hs}
                                es = {h: esp.tile([128, 512], F32R, tag="es",
                                                  name="es")
                                      for h in hs}
                                exq = {}
                                for kb in range(nkb + PIPE):
                                    if kb < nkb:
                                        klhs = acc[ik][:, kb * 128:(kb + 1) * 128]
                                        for h in hs:
                                            ps = spp.tile([128, 512], F32,
                                                          tag="ps", name="ps")
                                            nc.tensor.matmul(
                                                ps[:], klhs, acc[h][:, qsl],
                                                start=True, stop=True)
                                            if kb >= 4 * J:
                                                nc.vector.tensor_tensor(
                                                    ps[:], ps[:],
                                                    sb_mask[:, kb - 4 * J, :],
                                                    op=ADD)
                                            ex = exps.tile([128, 512], F32R,
                                                           tag="ex", name="ex")
                                            nc.scalar.activation(ex[:], ps[:],
                                                                 EXP,
                                                                 scale=SCALE)
                                            exq[(kb, h)] = ex
                                    kbd = kb - PIPE
                                    if kbd < 0 or kbd >= nkb:
                                        continue
                                    st = (kbd == 0)
                                    sp = (kbd == nkb - 1)
                                    for h in hs:
                                        nc.tensor.matmul(pa[h][:],
                                                         v_t[:, kbd, :],
                                                         exq[(kbd, h)][:],
                                                         start=st, stop=sp)
                                    for h in hs:
                                        ex = exq.pop((kbd, h))
                                        if st:
                                            nc.vector.tensor_copy(es[h][:],
                                                                  ex[:])
                                        else:
                                            nc.vector.tensor_tensor(
                                                es[h][:], es[h][:], ex[:],
                                                op=ADD)
                                for h in hs:
                                    # cross-partition sum -> broadcast, then
                                    # reciprocal; result is already [128, 512]
                                    # so no partition-broadcast DMA is needed
                                    pr = esp.tile([128, 512], F32R, tag="pr",
                                                  name="pr")
                                    nc.gpsimd.partition_all_reduce(
                                        pr[:], es[h][:], channels=128,
                                        reduce_op=bass_isa.ReduceOp.add)
                                    rb = rcb.tile([128, 512], F32R, tag="rb",
                                                  name="rb")
                                    nc.vector.reciprocal(rb[:], pr[:])
                                    nc.vector.tensor_tensor(attnT[h][:, qsl],
                                                            pa[h][:],
                                                            rb[:], op=MULT)

                    rope_ctx.close()

                    # ------ Phase 3: partial out projection (fp16) ------
                    # Weight-stationary: one w_o block feeds all 4 t-quads.
                    # Partials land in a DRAM buffer, then on-device
                    # ReduceScatter (Shared output) sums them and leaves this
                    # core's [xsh, t] shard for the tiny output DMA.
                    if use_cc:
                        # two half-partials: RS of the first half overlaps the
                        # PE work of the second half of the o-loop. Core c's
                        # output shard is [global rows 384c..384c+384] ++
                        # [3072+384c..3072+384c+384]; the host undoes this.
                        hd2 = d_model // 2
                        part = [dram.tile([hd2, t], F16, tag=f"part{h}",
                                          name=f"part{h}_r{rep}")
                                for h in range(2)]
                        rs_out = [dram.tile([xsh // 2, t], F16, tag=f"rs{h}",
                                            name=f"rs{h}_r{rep}")
                                  for h in range(2)]
                    with ExitStack() as ctx:
                        wop = ctx.enter_context(tc.tile_pool(name="wop",
                                                             bufs=4))
                        outp = ctx.enter_context(tc.tile_pool(name="outp",
                                                              bufs=2 * tq))
                        pso = ctx.enter_context(
                            tc.tile_pool(name="pso", bufs=2 * tq, space="PSUM"))
                        for o in range(d_model // 128):
                            wo = wop.tile([128, nqh, 128], F16, tag="wo")
                            # w_o[s*128+p, o*128+oo] lives at flat off["wo"] +
                            # 6144*(s*128+p) + 128*o + oo
                            nc.sync.dma_start(
                                wo[:], pk_ap(off["wo"] + o * 128,
                                             [(d_model, 128),
                                              (d_model * 128, nqh),
                                              (1, 128)]))
                            pos = [pso.tile([128, 512], F32, tag="po",
                                            name=f"po{q}")
                                   for q in range(tq)]
                            for s in range(nqh):
                                lhs = wo[:, s, :]
                                for J in range(tq):
                                    nc.tensor.matmul(
                                        pos[J][:], lhs,
                                        attnT[s][:, J * 512:(J + 1) * 512],
                                        start=(s == 0), stop=(s == nqh - 1))
                            for J in range(tq):
                                ob = outp.tile([128, 512], F16, tag="ob",
                                               name="ob")
                                nc.vector.tensor_copy(ob[:], pos[J][:])
                                if use_cc:
                                    dst = (part[0] if o * 128 < d_model // 2
                                           else part[1])
                                    ro = o * 128 - (0 if o * 128 < d_model // 2
                                                    else d_model // 2)
                                else:
                                    dst, ro = outT, o * 128
                                nc.sync.dma_start(
                                    dst[ro:ro + 128, J * 512:(J + 1) * 512],
                                    ob[:])

                    if use_cc:
                        for h in range(2):
                            if "rs" not in _skip:
                                nc.gpsimd.collective_compute(
                                    "ReduceScatter", ADD,
                                    replica_groups=[list(range(n_cores))],
                                    ins=[part[h].opt()], outs=[rs_out[h].opt()])
                                nc.sync.dma_start(
                                    outT[h * (xsh // 2):(h + 1) * (xsh // 2), :],
                                    rs_out[h][:])
                            else:
                                # timing-only: skip the collective, emit the
                                # same-size final DMA from the partial buffer
                                nc.sync.dma_start(
                                    outT[h * (xsh // 2):(h + 1) * (xsh // 2), :],
                                    part[h][:xsh // 2, :])

            if use_cc:
                gathered = {0: emit_gather(0)}

                def make_next(r):
                    def cb():
                        gathered[r] = emit_gather(r)
                    return cb

                for rep in range(reps):
                    emit_rep(rep, gathered.pop(rep),
                             make_next(rep + 1) if rep + 1 < reps else None)
            else:
                for rep in range(reps):
                    emit_rep(rep, None, None)

    with tile.TileContext(nc) as tc, nc.allow_low_precision(reason="fp16/fp32r matmuls"):
        _body(tc)
    nc.compile()
    return nc


def make_core_inputs(x, causal_mask, w_qkv, w_out, d_model=D_MODEL, t=T, nqh=NQH,
                     n_cores=N_CORES, use_cc=True):
    """Host-side sharding: per-core input dicts for the SPMD program."""
    x2 = np.ascontiguousarray(x.reshape(t, d_model).T).astype(np.float16)  # [D, T]
    kv_base = nqh * n_cores * HEAD_DIM
    n_kv = n_cores
    xsh = d_model // n_cores

    # RoPE tables in [head_dim, T] layout (interleaved-pair convention).
    hd = HEAD_DIM
    inv = 1.0 / ROPE_BASE ** (np.arange(0, hd, 2, dtype=np.float64) / hd)
    pos = np.arange(t, dtype=np.float64)
    freqs = pos[None, :] * inv[:, None]                     # [hd/2, T]
    cos = np.cos(freqs)
    sin = np.sin(freqs)
    cosf = np.empty((hd, t), dtype=np.float16)
    sinf = np.empty((hd, t), dtype=np.float16)
    cosf[0::2] = cos
    cosf[1::2] = cos
    sinf[0::2] = -sin                                       # row 2i:   -sin
    sinf[1::2] = sin                                        # row 2i+1: +sin

    # causal mask diagonal-quad slices, clipped to an fp16-safe big-negative
    cm = causal_mask.reshape(causal_mask.shape[-2], causal_mask.shape[-1])
    cm = np.maximum(np.asarray(cm, dtype=np.float32), MASK_NEG)
    maskT = np.stack([np.ascontiguousarray(cm[0:512, m * 128:(m + 1) * 128].T)
                      for m in range(4)]).astype(np.float16)

    pswap = np.zeros((128, 128), dtype=np.float16)
    for i in range(0, 128, 2):
        pswap[i, i + 1] = 1.0
        pswap[i + 1, i] = 1.0
    ident = np.eye(128, dtype=np.float16)
    ones = np.ones(128, dtype=np.float16)

    in_maps = []
    for c in range(n_cores):
        qrows = np.arange(c * nqh * 128, (c + 1) * nqh * 128)
        krows = np.arange(kv_base + c * 128, kv_base + (c + 1) * 128)
        vrows = np.arange(kv_base + n_kv * 128 + c * 128,
                          kv_base + n_kv * 128 + (c + 1) * 128)
        rows = np.concatenate([qrows, krows, vrows])
        wqkvT_c = np.ascontiguousarray(w_qkv[rows, :].T).astype(np.float16)
        w_o_c = np.ascontiguousarray(w_out[:, qrows].T).astype(np.float16)
        xs_c = (np.ascontiguousarray(x2[c * xsh:(c + 1) * xsh, :])
                if use_cc else x2)
        pk = np.concatenate([xs_c.ravel(), wqkvT_c.ravel(), w_o_c.ravel(),
                             cosf.ravel(), sinf.ravel(), maskT.ravel(),
                             pswap.ravel(), ident.ravel(), ones])
        in_maps.append({"pk": pk})
    return in_maps


_PROGRAM_CACHE = {}


def _get_program(reps=1):
    key = (D_MODEL, T, NQH, reps)
    if key not in _PROGRAM_CACHE:
        _PROGRAM_CACHE[key] = build_program(reps=reps)
    return _PROGRAM_CACHE[key]


def kernel(x, causal_mask, w_qkv, w_out):
    x = np.asarray(x, dtype=np.float32)
    causal_mask = np.asarray(causal_mask, dtype=np.float32)
    w_qkv = np.asarray(w_qkv, dtype=np.float32)
    w_out = np.asarray(w_out, dtype=np.float32)

    nc = _get_program()
    in_maps = make_core_inputs(x, causal_mask, w_qkv, w_out)
    res = run_bass_kernel_spmd(nc, in_maps, list(range(N_CORES)))
    shards = [np.asarray(res.results[c]["outT"], dtype=np.float32)
              for c in range(N_CORES)]
    h = XSH // 2
    top = np.concatenate([s[:h] for s in shards], axis=0)   # rows 0..D/2
    bot = np.concatenate([s[h:] for s in shards], axis=0)   # rows D/2..D
    outT = np.concatenate([top, bot], axis=0)               # [D, T]
    return np.ascontiguousarray(outT.T).reshape(1, T, D_MODEL).astype(np.float32)
